# revision 9
# baseline (speedup 1.0000x reference)
"""Trainium2 Bass kernel for nn_HGRanking (GNN message passing).

Strategy (8 NeuronCores, SPMD, no collectives):
  - Shard by graphs: 64 contiguous graphs per core (segment_ids is sorted, so
    each core's nodes are one contiguous range, padded to NP).
  - Embedding gather on-device via indirect DMA from the (host-fp16-cast) table.
  - Node features stay SBUF-resident in a transposed fp16 layout
    XT[c] = [128 feat-partitions, 4 feat-tiles, 512 nodes]; all matmuls
    accumulate in fp32 PSUM.
  - segment_sum  == matmul with a one-hot (segs x nodes) matrix.
    s[seg]-broadcast == extra K-tile in the layer-1 accumulation using the
    transposed one-hot, so it is free of gather/scatter entirely.
  - The mp_round-th node update is dead code (output depends only on the final
    set state), so only (mp_round-1) node updates are executed; the next
    round's segment-sum is fused into each chunk's pipeline via PE transposes.
"""

import numpy as np

H = 512
B = 512
NCORES = 8
GPC = B // NCORES  # graphs per core = 64
V = 32000


def _build_program(NP, R, reps=1):
    import concourse.bass as bass
    import concourse.mybir as mybir
    import concourse.tile as tile
    from concourse import bacc
    from concourse.masks import make_identity
    from contextlib import ExitStack

    f16 = mybir.dt.float16
    f32 = mybir.dt.float32
    i32 = mybir.dt.int32
    AF = mybir.ActivationFunctionType
    ts, ds = bass.ts, bass.ds

    NSUB = NP // 128
    NCH = NP // 512

    nc = bacc.Bacc("TRN2")

    emb_d = nc.dram_tensor("emb16", [V, H], f16, kind="ExternalInput")
    idx_d = nc.dram_tensor("idx", [128, NSUB], i32, kind="ExternalInput")
    ohT_d = nc.dram_tensor("onehotT", [128, NSUB, GPC], f16, kind="ExternalInput")
    oh64_d = nc.dram_tensor("onehot64", [GPC, NP], f16, kind="ExternalInput")
    w1top_d = nc.dram_tensor("w1top", [H, H], f16, kind="ExternalInput")
    w1bot_d = nc.dram_tensor("w1bot", [H, H], f16, kind="ExternalInput")
    w2n_d = nc.dram_tensor("w2n", [H, H], f16, kind="ExternalInput")
    sw1_d = nc.dram_tensor("sw1", [2 * H, H], f16, kind="ExternalInput")
    sw2_d = nc.dram_tensor("sw2", [H, H], f16, kind="ExternalInput")
    rw1_d = nc.dram_tensor("rw1", [H, H], f16, kind="ExternalInput")
    rw2_d = nc.dram_tensor("rw2", [H, H], f16, kind="ExternalInput")
    bias_d = nc.dram_tensor("biases", [128, 24], f32, kind="ExternalInput")
    out_d = nc.dram_tensor("out", [1, GPC], f32, kind="ExternalOutput")

    with tile.TileContext(nc) as tc, ExitStack() as ctx:
        const_p = ctx.enter_context(tc.tile_pool(name="const", bufs=1))
        xt_p = ctx.enter_context(tc.tile_pool(name="xt", bufs=1))
        small_p = ctx.enter_context(tc.tile_pool(name="small", bufs=2))
        h1_p = ctx.enter_context(tc.tile_pool(name="h1", bufs=2))
        gx_p = ctx.enter_context(tc.tile_pool(name="gx", bufs=4))
        ps_h1 = ctx.enter_context(tc.tile_pool(name="ps_h1", bufs=2, space="PSUM"))
        ps_l2 = ctx.enter_context(tc.tile_pool(name="ps_l2", bufs=2, space="PSUM"))
        ps_x = ctx.enter_context(tc.tile_pool(name="ps_x", bufs=2, space="PSUM"))
        ps_agg = ctx.enter_context(tc.tile_pool(name="ps_agg", bufs=2, space="PSUM"))

        # ---- constants into SBUF ----
        def kload(name, dram, ktiles):
            t = const_p.tile([128, ktiles, H], f16, tag=name, name=name)
            nc.sync.dma_start(out=t[:], in_=dram[:].rearrange("(k p) m -> p k m", p=128))
            return t

        w1top_sb = kload("w1top_sb", w1top_d, 4)
        w1bot_sb = kload("w1bot_sb", w1bot_d, 4)
        w2n_sb = kload("w2n_sb", w2n_d, 4)
        sw1_sb = kload("sw1_sb", sw1_d, 8)
        sw2_sb = kload("sw2_sb", sw2_d, 4)
        rw1_sb = kload("rw1_sb", rw1_d, 4)
        rw2_sb = kload("rw2_sb", rw2_d, 4)

        ohT_sb = const_p.tile([128, NSUB, GPC], f16, tag="ohT", name="ohT_sb")
        nc.sync.dma_start(out=ohT_sb[:], in_=ohT_d[:])
        oh64_sb = const_p.tile([GPC, NP], f16, tag="oh64", name="oh64_sb")
        nc.sync.dma_start(out=oh64_sb[:], in_=oh64_d[:])
        idx_sb = const_p.tile([128, NSUB], i32, tag="idx", name="idx_sb")
        nc.sync.dma_start(out=idx_sb[:], in_=idx_d[:])
        bias_sb = const_p.tile([128, 24], f32, tag="bias", name="bias_sb")
        nc.sync.dma_start(out=bias_sb[:], in_=bias_d[:])

        def bcol(group, m):  # per-partition bias column [128, 1]
            return bias_sb[:, group * 4 + m : group * 4 + m + 1]

        ident = const_p.tile([128, 128], f16, tag="ident", name="ident")
        make_identity(nc, ident[:])
        ones_sb = const_p.tile([128, 1], f32, tag="ones", name="ones_sb")
        nc.gpsimd.memset(ones_sb[:], 1.0)

        xt = [
            xt_p.tile([128, 4, 512], f16, tag=f"xt{c}", name=f"xt{c}")
            for c in range(NCH)
        ]

        for rep in range(reps):
            _emit_body(
                nc, tc, mybir, bass, NP, R, rep,
                small_p, h1_p, gx_p, ps_h1, ps_l2, ps_x, ps_agg,
                xt, emb_d, idx_sb, ohT_sb, oh64_sb, ident, ones_sb, bcol,
                w1top_sb, w1bot_sb, w2n_sb, sw1_sb, sw2_sb, rw1_sb, rw2_sb,
                out_d,
            )

    nc.compile()
    return nc


def _emit_body(
    nc, tc, mybir, bass, NP, R, rep,
    small_p, h1_p, gx_p, ps_h1, ps_l2, ps_x, ps_agg,
    xt, emb_d, idx_sb, ohT_sb, oh64_sb, ident, ones_sb, bcol,
    w1top_sb, w1bot_sb, w2n_sb, sw1_sb, sw2_sb, rw1_sb, rw2_sb,
    out_d,
):
    f16 = mybir.dt.float16
    f32 = mybir.dt.float32
    AF = mybir.ActivationFunctionType
    ts, ds = bass.ts, bass.ds
    NSUB = NP // 128
    NCH = NP // 512
    P = f"q{rep}_"

    if True:
        # s_0 = ones, stored transposed: [128 featpart, 4 feattile, 64 segs]
        sT = small_p.tile([128, 4, GPC], f16, tag="sT", name=P + "sT0")
        nc.gpsimd.memset(sT[:], 1.0)

        # ---- init: gather emb rows -> X subchunks; agg_1; build XT ----
        aggp = None
        if R > 0:
            aggp = ps_agg.tile([GPC, H], f32, tag="ps_agg", name=P + "aggp_init")
            for j in range(NSUB):
                gbuf = gx_p.tile([128, H], f16, tag="gx", name=P + f"gx{j}")
                nc.gpsimd.indirect_dma_start(
                    out=gbuf[:],
                    out_offset=None,
                    in_=emb_d[:],
                    in_offset=bass.IndirectOffsetOnAxis(ap=idx_sb[:, j : j + 1], axis=0),
                )
                nc.tensor.matmul(
                    out=aggp[:],
                    lhsT=ohT_sb[:, j, :],
                    rhs=gbuf[:],
                    start=(j == 0),
                    stop=(j == NSUB - 1),
                )
                if R > 1:
                    xp = ps_x.tile([128, 4, 128], f16, tag="ps_x", name=P + f"xpi{j}")
                    for k in range(4):
                        nc.tensor.transpose(
                            out=xp[:, k, :], in_=gbuf[:, ts(k, 128)], identity=ident[:]
                        )
                    c_, sub = j // 4, j % 4
                    nc.vector.tensor_copy(
                        xt[c_][:, :, ds(sub * 128, 128)], xp[:]
                    )

        # ---- rounds ----
        for r in range(R):
            last = r == R - 1
            # set-MLP: s_{r+1} = relu(relu([agg, s] @ sw1 + sb1) @ sw2 + sb2)
            agg_sb = small_p.tile([GPC, H], f16, tag="aggsb", name=f"aggsb{r}")
            nc.vector.tensor_copy(agg_sb[:], aggp[:])
            aggT = small_p.tile([128, 4, GPC], f16, tag="aggT", name=f"aggT{r}")
            for m in range(4):
                tp = ps_h1.tile([128, GPC], f16, tag="ps_h1", name=f"tp{r}_{m}")
                nc.tensor.transpose(
                    out=tp[:], in_=agg_sb[:, ts(m, 128)], identity=ident[:GPC, :GPC]
                )
                nc.vector.tensor_copy(aggT[:, m, :], tp[:])
            g1T = small_p.tile([128, 4, GPC], f16, tag="g1T", name=f"g1T{r}")
            for m in range(4):
                gp = ps_h1.tile([128, GPC], f32, tag="ps_h1", name=f"gp{r}_{m}")
                for k in range(8):
                    rhs = aggT[:, k, :] if k < 4 else sT[:, k - 4, :]
                    nc.tensor.matmul(
                        out=gp[:],
                        lhsT=sw1_sb[:, k, ts(m, 128)],
                        rhs=rhs,
                        start=(k == 0),
                        stop=(k == 7),
                    )
                nc.scalar.activation(g1T[:, m, :], gp[:], AF.Relu, bias=bcol(2, m))
            sT_new = small_p.tile([128, 4, GPC], f16, tag="sT", name=f"sT{r + 1}")
            for m in range(4):
                sp = ps_h1.tile([128, GPC], f32, tag="ps_h1", name=f"sp{r}_{m}")
                for k in range(4):
                    nc.tensor.matmul(
                        out=sp[:],
                        lhsT=sw2_sb[:, k, ts(m, 128)],
                        rhs=g1T[:, k, :],
                        start=(k == 0),
                        stop=(k == 3),
                    )
                nc.scalar.activation(sT_new[:, m, :], sp[:], AF.Relu, bias=bcol(3, m))
            sT = sT_new
            if last:
                break

            # c = s_{r+1} @ W1bot  (normal layout [seg, feat])
            cp = ps_l2.tile([GPC, H], f32, tag="ps_l2", name=f"cp{r}")
            for k in range(4):
                nc.tensor.matmul(
                    out=cp[:],
                    lhsT=sT[:, k, :],
                    rhs=w1bot_sb[:, k, :],
                    start=(k == 0),
                    stop=(k == 3),
                )
            c_sb = small_p.tile([GPC, H], f16, tag="csb", name=f"csb{r}")
            nc.vector.tensor_copy(c_sb[:], cp[:])

            # node MLP over chunks of 512 nodes, with fused next-round agg
            aggp_next = ps_agg.tile([GPC, H], f32, tag="ps_agg", name=f"aggp{r + 1}")
            for c in range(NCH):
                h1c = h1_p.tile([128, 4, H], f16, tag="h1c", name=f"h1c_{r}_{c}")
                for m in range(4):
                    hp = ps_h1.tile([128, H], f32, tag="ps_h1", name=f"hp{r}_{c}_{m}")
                    for k in range(4):
                        nc.tensor.matmul(
                            out=hp[:],
                            lhsT=w1top_sb[:, k, ts(m, 128)],
                            rhs=xt[c][:, k, :],
                            start=(k == 0),
                            stop=False,
                        )
                    nc.tensor.matmul(
                        out=hp[:],
                        lhsT=c_sb[:, ts(m, 128)],
                        rhs=oh64_sb[:, ds(c * 512, 512)],
                        start=False,
                        stop=True,
                    )
                    nc.scalar.activation(h1c[:, m, :], hp[:], AF.Relu, bias=bcol(0, m))
                for m in range(4):
                    lp = ps_l2.tile([128, H], f32, tag="ps_l2", name=f"lp{r}_{c}_{m}")
                    for k in range(4):
                        nc.tensor.matmul(
                            out=lp[:],
                            lhsT=w2n_sb[:, k, ts(m, 128)],
                            rhs=h1c[:, k, :],
                            start=(k == 0),
                            stop=(k == 3),
                        )
                    nc.scalar.activation(
                        xt[c][:, m, :], lp[:], AF.Relu, bias=bcol(1, m)
                    )
                # fused segment-sum for next round: transpose back + one-hot MM
                for sub in range(4):
                    j = c * 4 + sub
                    xp = ps_x.tile([128, 4, 128], f16, tag="ps_x", name=f"xp{r}_{j}")
                    for k in range(4):
                        nc.tensor.transpose(
                            out=xp[:, k, :],
                            in_=xt[c][:, k, ds(sub * 128, 128)],
                            identity=ident[:],
                        )
                    xsb = gx_p.tile([128, 4, 128], f16, tag="xsb", name=f"xsb{r}_{j}")
                    nc.vector.tensor_copy(xsb[:], xp[:])
                    nc.tensor.matmul(
                        out=aggp_next[:],
                        lhsT=ohT_sb[:, j, :],
                        rhs=xsb[:],
                        start=(j == 0),
                        stop=(j == NSUB - 1),
                    )
            aggp = aggp_next

        # ---- readout: norm(tanh(relu(s @ rw1 + rb1) @ rw2 + rb2)) ----
        rog = small_p.tile([128, 4, GPC], f16, tag="rog", name="rog")
        for m in range(4):
            rp = ps_h1.tile([128, GPC], f32, tag="ps_h1", name=f"rp{m}")
            for k in range(4):
                nc.tensor.matmul(
                    out=rp[:],
                    lhsT=rw1_sb[:, k, ts(m, 128)],
                    rhs=sT[:, k, :],
                    start=(k == 0),
                    stop=(k == 3),
                )
            nc.scalar.activation(rog[:, m, :], rp[:], AF.Relu, bias=bcol(4, m))
        oT = small_p.tile([128, 4, GPC], f32, tag="oT", name="oT")
        for m in range(4):
            op = ps_l2.tile([128, GPC], f32, tag="ps_l2", name=f"op{m}")
            for k in range(4):
                nc.tensor.matmul(
                    out=op[:],
                    lhsT=rw2_sb[:, k, ts(m, 128)],
                    rhs=rog[:, k, :],
                    start=(k == 0),
                    stop=(k == 3),
                )
            nc.scalar.activation(oT[:, m, :], op[:], AF.Tanh, bias=bcol(5, m))
        sq = small_p.tile([128, 4, GPC], f32, tag="sq", name="sq")
        nc.vector.tensor_tensor(
            out=sq[:], in0=oT[:], in1=oT[:], op=mybir.AluOpType.mult
        )
        np_ps = ps_agg.tile([1, GPC], f32, tag="ps_agg", name="np_ps")
        for m in range(4):
            nc.tensor.matmul(
                out=np_ps[:],
                lhsT=ones_sb[:, :1],
                rhs=sq[:, m, :],
                start=(m == 0),
                stop=(m == 3),
            )
        res = small_p.tile([1, GPC], f32, tag="res", name="res")
        nc.scalar.activation(res[:], np_ps[:], AF.Sqrt)
        nc.sync.dma_start(out=out_d[:], in_=res[:])


def _prepare(inputs):
    """Shard/pad the full inputs and build the SPMD program.

    Returns (nc, in_maps) — one compiled-ready Bass program plus the
    per-core input maps.
    """
    node_ids = np.asarray(inputs["node_ids"]).astype(np.int64).reshape(-1)
    seg = np.asarray(inputs["segment_ids"]).astype(np.int64).reshape(-1)
    R = int(np.asarray(inputs["mp_round"]))
    f32 = np.float32
    f16 = np.float16

    emb16 = np.ascontiguousarray(np.asarray(inputs["emb"], dtype=f32).astype(f16))
    w1 = np.asarray(inputs["node_w1"], dtype=f32)
    weights = {
        "w1top": np.ascontiguousarray(w1[:H].astype(f16)),
        "w1bot": np.ascontiguousarray(w1[H:].astype(f16)),
        "w2n": np.asarray(inputs["node_w2"], dtype=f32).astype(f16),
        "sw1": np.asarray(inputs["set_w1"], dtype=f32).astype(f16),
        "sw2": np.asarray(inputs["set_w2"], dtype=f32).astype(f16),
        "rw1": np.asarray(inputs["ro_w1"], dtype=f32).astype(f16),
        "rw2": np.asarray(inputs["ro_w2"], dtype=f32).astype(f16),
    }
    biases = np.zeros((128, 24), f32)
    for g, key in enumerate(["node_b1", "node_b2", "set_b1", "set_b2", "ro_b1", "ro_b2"]):
        vec = np.asarray(inputs[key], dtype=f32).reshape(-1)
        biases[:, g * 4 : (g + 1) * 4] = vec.reshape(4, 128).T

    # per-core contiguous node ranges (segment_ids sorted)
    bounds = np.searchsorted(seg, np.arange(0, B + 1, GPC))
    counts = np.diff(bounds)
    NP = int(max(8704, -(-int(counts.max()) // 512) * 512))
    NSUB = NP // 128

    in_maps = []
    for core in range(NCORES):
        lo, hi = int(bounds[core]), int(bounds[core + 1])
        n = hi - lo
        ids = np.zeros(NP, np.int64)
        ids[:n] = node_ids[lo:hi]
        lseg = np.full(NP, -1, np.int64)
        lseg[:n] = seg[lo:hi] - core * GPC
        assert n == 0 or (lseg[:n].min() >= 0 and lseg[:n].max() < GPC)

        idx_np = np.ascontiguousarray(ids.reshape(NSUB, 128).T.astype(np.int32))
        i_all = np.arange(NP)
        valid = lseg >= 0
        ohT = np.zeros((128, NSUB, GPC), f16)
        ohT[i_all[valid] % 128, i_all[valid] // 128, lseg[valid]] = 1
        oh64 = np.zeros((GPC, NP), f16)
        oh64[lseg[valid], i_all[valid]] = 1

        in_maps.append(
            dict(
                emb16=emb16,
                idx=idx_np,
                onehotT=ohT,
                onehot64=oh64,
                biases=biases,
                **weights,
            )
        )

    nc = _build_program(NP, R)
    return nc, in_maps


def kernel(**inputs):
    from concourse.bass_utils import run_bass_kernel_spmd

    nc, in_maps = _prepare(inputs)
    res = run_bass_kernel_spmd(nc, in_maps, list(range(NCORES)))
    out = np.concatenate(
        [np.asarray(res.results[c]["out"]).reshape(-1) for c in range(NCORES)]
    )
    return out.astype(np.float32)
